# revision 13
# baseline (speedup 1.0000x reference)
"""nn_CPQuadRankLayer kernel for 8x TRN2 NeuronCores.

Sharding: num_nodes (N=1024) split across 8 cores (128 nodes/core);
all per-node factor tensors sharded the same way (expert-parallel, no
collectives). Host marshalling is layout-only (reshape/transpose/byte
gather); all arithmetic happens on-device.

Per node n (B=32, IN=OUT=256, R=32):
  res   = mean_c x[b,n,c,:]
  xn    = LN(x) * gamma + beta
  p_c   = xn_c @ f_c^T                  (4 projections, [b,r])
  m     = scale * p_tl*p_tr*p_bl*p_br
  out   = m @ f_out + res

Design: DMA in 32-node superchunks; x and factor tensors marshalled as
bf16 (high 2 bytes of each fp32); factors+f_out fused into one DMA per
chunk, constants fused into one bf16 blob (fewer HWDGE issue slots);
pair-batched bn_stats; one sqrt + reciprocal per chunk; normalizes
split across ACT/GpSimd/DVE; bf16 PE transposes with one grouped PSUM
evacuation; packed 128x32-tiled stage-1 matmuls; Hadamard on bf16 with
the CP scale folded into a per-partition tensor_scalar (no f_out
pre-scale pass); residual mean as bf16 matmuls (constant 0.25 selector
lhsT) accumulated into the stage-2 PSUM; bf16 output (host casts to
fp32 at gather).

Per-group (4 nodes) device mapping, partitions = (c,b) for x:
  - 16x paired bn_stats + 32x bn_aggr -> mean/var per (c,b) (chunk)
  - ACT: sd=sqrt(var+eps) (chunk); DVE: rs=1/sd (chunk)
  - normalize xn = (x - mu) * rs (bf16): 1 ACT, 1 GpSimd, 2 DVE
  - 8x PE transpose (bf16) -> [i, (c,b)]; one grouped ACT evac
  - 32x bf16 matmul [k=128i, m=32r, n=32b] tiled (0,32q) -> ps1[qr, cb]
  - ACT evac ps1 (bf16); DVE: ta=tl*tr (*scale), tb=bl*br, 4x diag ->
    mdiag[qr, qb] (block-diag stage-2 lhsT, off-diag pre-zeroed)
  - 4x bf16 residual matmul + 1 bf16 stage-2 matmul (rhs = f_out)
    accumulate into ps2[(q,b), o]; ACT evac bf16; 1 output DMA/chunk
"""

import os

import numpy as np
import ml_dtypes
from contextlib import ExitStack

import concourse.bass as bass
import concourse.bacc as bacc
import concourse.tile as tile
import concourse.mybir as mybir
from concourse.bass_utils import run_bass_kernel_spmd

F32 = mybir.dt.float32
BF16 = mybir.dt.bfloat16
AF = mybir.ActivationFunctionType
ALU = mybir.AluOpType

B, N, IN_DIM, OUT_DIM, RANK = 32, 1024, 256, 256, 32
LN_EPS = 1e-5
N_CORES = 8
NL = N // N_CORES      # nodes per core = 128
NG = 4                 # nodes per group (PSUM stripe packing)
SC = 32                # nodes per superchunk (DMA granularity)
NCHUNK = NL // SC      # 8 superchunks per core
GPC = SC // NG         # groups per chunk = 4
FFW = SC * 2 * 128 + GPC * 256   # fused factors width = 9216


def build_program(nl=NL, affine=False):
    nc = bacc.Bacc("TRN2", target_bir_lowering=False, debug=False,
                   num_devices=N_CORES)

    xh_d = nc.dram_tensor("xh", [NCHUNK, 128, SC, 256], BF16,
                          kind="ExternalInput").ap()
    # fused per-chunk factor blob: [fth (SC*2*128) | foh (GPC*256)]
    ffh_d = nc.dram_tensor("ffh", [NCHUNK, 128, FFW], BF16,
                           kind="ExternalInput").ap()
    sc_d = nc.dram_tensor("sc", [128, nl // NG], F32,
                          kind="ExternalInput").ap()
    # fused bf16 constants: [smat (4*128) | idn (128)]
    cst_d = nc.dram_tensor("cst", [128, 5 * 128], BF16,
                           kind="ExternalInput").ap()
    gam_d = nc.dram_tensor("gam", [128, 2], F32, kind="ExternalInput").ap()
    bet_d = nc.dram_tensor("bet", [128, 2], F32, kind="ExternalInput").ap()
    oh_d = nc.dram_tensor("oh", [NCHUNK, 128, GPC, 256], BF16,
                          kind="ExternalOutput").ap()

    with tile.TileContext(nc) as tc, ExitStack() as ctx:
        cpool = ctx.enter_context(tc.tile_pool(name="const", bufs=1))
        px = ctx.enter_context(tc.tile_pool(name="px", bufs=2))
        pff = ctx.enter_context(tc.tile_pool(name="pff", bufs=2))
        pout = ctx.enter_context(tc.tile_pool(name="pout", bufs=2))
        pxn = ctx.enter_context(tc.tile_pool(name="pxn", bufs=12))
        pxbt = ctx.enter_context(tc.tile_pool(name="pxbt", bufs=6))
        pstat = ctx.enter_context(tc.tile_pool(name="pstat", bufs=4))
        pm = ctx.enter_context(tc.tile_pool(name="pm", bufs=4))
        pps_t = ctx.enter_context(tc.tile_pool(name="ps_t", bufs=3,
                                               space="PSUM"))
        pps1 = ctx.enter_context(tc.tile_pool(name="ps1", bufs=2,
                                              space="PSUM"))
        pps2 = ctx.enter_context(tc.tile_pool(name="ps2", bufs=3,
                                              space="PSUM"))

        # constants (2 DMAs)
        sc_sb = cpool.tile([128, nl // NG], F32, tag="sc")
        nc.sync.dma_start(out=sc_sb[:], in_=sc_d[:])
        cst_sb = cpool.tile([128, 5 * 128], BF16, tag="cst")
        nc.sync.dma_start(out=cst_sb[:], in_=cst_d[:])
        idn_ap = cst_sb[:, 4 * 128:5 * 128]
        eps_sb = cpool.tile([128, 1], F32, tag="eps")
        nc.vector.memset(eps_sb[:], LN_EPS)
        if affine:
            gam_sb = cpool.tile([128, 2], F32, tag="gam")
            nc.sync.dma_start(out=gam_sb[:], in_=gam_d[:])
            bet_sb = cpool.tile([128, 2], F32, tag="bet")
            nc.sync.dma_start(out=bet_sb[:], in_=bet_d[:])

        # pre-zeroed block-diag stage-2 lhsT slots (diag blocks rewritten
        # per group; off-diag stays zero for the whole kernel)
        md0 = cpool.tile([128, 128], BF16, tag="md0")
        md1 = cpool.tile([128, 128], BF16, tag="md1")
        md2 = cpool.tile([128, 128], BF16, tag="md2")
        md3 = cpool.tile([128, 128], BF16, tag="md3")
        mds = [md0, md1, md2, md3]
        for md in mds:
            nc.vector.memset(md[:], 0.0)

        for t in range(NCHUNK):
            # sub-chunk input DMAs: stats (and then everything downstream)
            # start as soon as the first 8-node slice lands instead of
            # waiting for the whole 1MB transfer to round-robin through
            # the DMA queues.
            xg = px.tile([128, SC, 256], BF16, tag="xg")
            for u in range(4):
                nc.sync.dma_start(out=xg[:, 8 * u:8 * (u + 1)],
                                  in_=xh_d[t, :, 8 * u:8 * (u + 1)])
            ffg = pff.tile([128, FFW], BF16, tag="ffg")
            h = FFW // 2
            nc.sync.dma_start(out=ffg[:, :h], in_=ffh_d[t, :, :h])
            nc.sync.dma_start(out=ffg[:, h:], in_=ffh_d[t, :, h:])
            osb = pout.tile([128, GPC, 256], BF16, tag="osb")

            def ft_ap(j, k, c):
                o = (j * 2 + k) * 128 + 32 * c
                return ffg[:, o:o + 32]

            def fo_ap(gg):
                o = SC * 2 * 128 + gg * 256
                return ffg[:, o:o + 256]

            # LN stats, processed in two half-chunk waves so the first
            # groups' normalizes start before the whole chunk's stats are
            # done.  bn_stats emits per-node (cnt, mean, cnt*var) for two
            # equal 128-element interleaves; combine manually:
            #   mean = (m0+m1)/2
            #   var  = (cv0+cv1)/256 + (m0-m1)^2/4
            #   sd   = sqrt(((cv0+cv1)/64 + (m0-m1)^2) * 0.25 + eps)
            st = pstat.tile([128, SC, 6], F32, tag="st")
            mu = pstat.tile([128, SC], F32, tag="mu")
            rs = pstat.tile([128, SC], F32, tag="rs")
            nmurs = pstat.tile([128, SC], F32, tag="nmurs")
            HS = SC // 2
            for hh in range(2):
                sl = slice(HS * hh, HS * (hh + 1))
                for j in range(HS * hh, HS * (hh + 1)):
                    nc.vector.bn_stats(st[:, j], xg[:, j])
                nc.vector.tensor_tensor(mu[:, sl], st[:, sl, 1],
                                        st[:, sl, 4], op=ALU.add)
                nc.vector.tensor_scalar_mul(mu[:, sl], mu[:, sl], 0.5)
                dd = pstat.tile([128, HS], F32, tag="dd")
                nc.vector.tensor_tensor(dd[:], st[:, sl, 1], st[:, sl, 4],
                                        op=ALU.subtract)
                vin = pstat.tile([128, HS], F32, tag="vin")
                nc.vector.tensor_tensor(vin[:], st[:, sl, 2], st[:, sl, 5],
                                        op=ALU.add)
                nc.vector.tensor_scalar_mul(vin[:], vin[:], 1.0 / 64.0)
                nc.vector.tensor_tensor(dd[:], dd[:], dd[:], op=ALU.mult)
                nc.vector.tensor_tensor(vin[:], vin[:], dd[:], op=ALU.add)
                sd = pstat.tile([128, HS], F32, tag="sd")
                nc.scalar.activation(sd[:], vin[:], AF.Sqrt,
                                     bias=eps_sb[:], scale=0.25)
                nc.vector.reciprocal(rs[:, sl], sd[:])
                # -mu*rs bias rows for the ACT-side normalizes
                nc.vector.tensor_tensor(nmurs[:, sl], mu[:, sl], rs[:, sl],
                                        op=ALU.mult)
                nc.vector.tensor_scalar_mul(nmurs[:, sl], nmurs[:, sl],
                                            -1.0)

            for gg in range(GPC):
                g = GPC * t + gg          # global group id
                j0 = NG * gg              # first in-chunk node of group

                ps1 = pps1.tile([128, 128], F32, tag="ps1")
                ps2 = pps2.tile([128, 256], F32, tag="ps2")
                ps_t = pps_t.tile([128, NG, 2, 128], BF16, tag="ps_t")

                for q in range(NG):
                    j = j0 + q
                    # residual: 0.25 * sum_c x -> ps2[(q,b), :] (bf16)
                    nc.tensor.matmul(
                        ps2[:], lhsT=cst_sb[:, 128 * q:128 * (q + 1)],
                        rhs=xg[:, j],
                        start=(q == 0), stop=False, skip_group_check=True)

                    # normalize: engines balanced 2x ACT / 2x DVE
                    xnq = pxn.tile([128, 256], BF16, tag="xnq")
                    if q < 2:
                        nc.scalar.activation(xnq[:], xg[:, j], AF.Identity,
                                             bias=nmurs[:, j:j + 1],
                                             scale=rs[:, j:j + 1])
                    else:
                        nc.vector.tensor_scalar(
                            xnq[:], xg[:, j], mu[:, j:j + 1],
                            rs[:, j:j + 1],
                            op0=ALU.subtract, op1=ALU.mult)

                    # PE transpose -> [i, (c,b)] bf16
                    nc.tensor.transpose(ps_t[:, q, 0], xnq[:, 0:128],
                                        idn_ap)
                    nc.tensor.transpose(ps_t[:, q, 1], xnq[:, 128:256],
                                        idn_ap)

                # transpose evac: one ACT pass per group
                xbt = pxbt.tile([128, NG, 2, 128], BF16, tag="xbt")
                if affine:
                    for k in range(2):
                        nc.vector.tensor_scalar(
                            xbt[:, :, k], ps_t[:, :, k],
                            gam_sb[:, k:k + 1], bet_sb[:, k:k + 1],
                            op0=ALU.mult, op1=ALU.add)
                else:
                    nc.scalar.copy(xbt[:], ps_t[:])

                # stage-1: 8 bf16 matmuls per node -> ps1[32q:+32, (c,b)]
                for q in range(NG):
                    j = j0 + q
                    for c in range(4):
                        for k in range(2):
                            nc.tensor.matmul(
                                ps1[32 * q:32 * (q + 1), 32 * c:32 * (c + 1)],
                                lhsT=ft_ap(j, k, c),
                                rhs=xbt[:, q, k, 32 * c:32 * (c + 1)],
                                start=(k == 0), stop=(k == 1),
                                tile_position=(0, 32 * q))

                # Hadamard -> block-diag stage-2 lhsT (bf16 evac; the CP
                # scale is a per-partition (q,r) vector, folded into the
                # evacuation of the first c-block)
                pp = pm.tile([128, 128], BF16, tag="pp")
                nc.scalar.activation(pp[:, 0:32], ps1[:, 0:32], AF.Copy,
                                     scale=sc_sb[:, g:g + 1])
                nc.scalar.copy(pp[:, 32:128], ps1[:, 32:128])
                ta = pm.tile([128, 32], BF16, tag="ta")
                nc.vector.tensor_tensor(ta[:], pp[:, 0:32], pp[:, 32:64],
                                        op=ALU.mult)
                tb = pm.tile([128, 32], BF16, tag="tb")
                nc.vector.tensor_tensor(tb[:], pp[:, 64:96], pp[:, 96:128],
                                        op=ALU.mult)
                md = mds[g % 4]
                for q in range(NG):
                    nc.vector.tensor_tensor(
                        md[32 * q:32 * (q + 1), 32 * q:32 * (q + 1)],
                        ta[32 * q:32 * (q + 1), :],
                        tb[32 * q:32 * (q + 1), :], op=ALU.mult)

                # stage-2: ps2[(q,b), o] += md.T @ f_out (bf16)
                nc.tensor.matmul(ps2[:], lhsT=md[:], rhs=fo_ap(gg),
                                 start=False, stop=True,
                                 skip_group_check=True)
                nc.scalar.copy(osb[:, gg], ps2[:])

            # issue the output DMA from the ACT queue: the last osb write is
            # ACT's own copy, so this never blocks; keeping it off the Sync
            # queue lets the next chunks' input prefetches issue early
            # instead of queuing behind an output that waits on compute.
            nc.scalar.dma_start(out=oh_d[t], in_=osb[:])

    nc.compile()
    return nc


def _hi_bf16(a):
    """Layout-only fp32 -> bf16: take the high 2 bytes of each little-endian
    fp32 element (truncation rounding). No host arithmetic — the device
    consumes these tensors in bf16 anyway; this moves the (truncating)
    downcast into the shard-marshalling byte gather instead of burning DMA
    bandwidth + an on-device cast pass on mantissa bits the kernel discards.
    """
    a = np.ascontiguousarray(np.asarray(a, dtype=np.float32))
    return np.ascontiguousarray(a.view('<u2')[..., 1::2]).view(
        ml_dtypes.bfloat16)


def host_prep(inputs, nl=NL):
    """Layout-only host prep -> list of per-core input maps."""
    x = _hi_bf16(inputs["x"])
    f_all = np.stack([_hi_bf16(inputs["factor_tl"]),
                      _hi_bf16(inputs["factor_tr"]),
                      _hi_bf16(inputs["factor_bl"]),
                      _hi_bf16(inputs["factor_br"])], axis=0)  # [4,N,R,IN]
    f_out = _hi_bf16(inputs["factor_out"])
    scale = np.asarray(inputs["scale"], dtype=np.float32)
    gamma = np.asarray(inputs["ln_gamma"], dtype=np.float32)
    beta = np.asarray(inputs["ln_beta"], dtype=np.float32)
    affine = bool(np.any(gamma != 1.0) or np.any(beta != 0.0))

    smat = np.zeros((128, NG, 128), ml_dtypes.bfloat16)
    p = np.arange(128)
    for q in range(NG):
        smat[p, q, 32 * q + (p % 32)] = 0.25
    idn = np.eye(128, dtype=ml_dtypes.bfloat16)
    cst = np.concatenate([smat.reshape(128, 4 * 128), idn],
                         axis=1)                  # [128, 5*128]
    gam2 = np.ascontiguousarray(gamma.reshape(2, 128).T)
    bet2 = np.ascontiguousarray(beta.reshape(2, 128).T)

    maps = []
    for kcore in range(N_CORES):
        s0, s1 = kcore * nl, (kcore + 1) * nl
        xk = x[:, s0:s1]                       # [B=32, nl, 4, IN]
        # xh[t, c*32+b, j, i] = x[b, 32t+j, c, i]
        xh = np.ascontiguousarray(
            xk.reshape(32, NCHUNK, SC, 4, 256)
              .transpose(1, 3, 0, 2, 4)).reshape(NCHUNK, 128, SC, 256)
        ftk = f_all[:, s0:s1]                  # [4, nl, R, IN]
        # fth[t, p, j, k, c*32+r] = f[c, 32t+j, r, 128k+p]
        fth = np.ascontiguousarray(
            ftk.reshape(4, NCHUNK, SC, 32, 2, 128)
               .transpose(1, 5, 2, 4, 0, 3)).reshape(NCHUNK, 128, SC * 2 * 128)
        # foh[t, 32q+r, gg, o] = f_out[32t+4gg+q, r, o]
        foh = np.ascontiguousarray(
            f_out[s0:s1].reshape(NCHUNK, GPC, NG, 32, 256)
                        .transpose(0, 2, 3, 1, 4)).reshape(NCHUNK, 128,
                                                           GPC * 256)
        ffh = np.ascontiguousarray(
            np.concatenate([fth, foh], axis=2))  # [NCHUNK, 128, FFW]
        # sc[32q+r, G] = scale[4G+q, r]
        sck = np.ascontiguousarray(
            scale[s0:s1].reshape(nl // NG, NG, 32)
                        .transpose(1, 2, 0)).reshape(128, nl // NG)
        maps.append(dict(xh=xh, ffh=ffh, sc=sck, cst=cst,
                         gam=gam2, bet=bet2))
    return maps, affine


_CACHE = {}
LAST_EXEC_NS = None


def kernel(**inputs) -> np.ndarray:
    global LAST_EXEC_NS
    maps, affine = host_prep(inputs)
    if affine not in _CACHE:
        _CACHE[affine] = build_program(NL, affine)
    nc = _CACHE[affine]

    trace = bool(int(os.environ.get("KTRACE", "0")))
    tmpdir = os.environ.get("KTRACE_DIR") or None
    res = run_bass_kernel_spmd(nc, maps, list(range(N_CORES)),
                               trace=trace, tmpdir=tmpdir)
    LAST_EXEC_NS = res.exec_time_ns
    outs = []
    for kcore in range(N_CORES):
        o = res.results[kcore]["oh"]           # [NCHUNK, 128, GPC, 256] bf16
        # o[t, 32q+b, gg, i] -> out[b, 32t+4gg+q, i]
        ok = np.asarray(o).astype(np.float32)
        ok = ok.reshape(NCHUNK, NG, 32, GPC, 256).transpose(2, 0, 3, 1, 4)
        outs.append(np.ascontiguousarray(ok).reshape(32, NL, 256))
    return np.concatenate(outs, axis=1)        # [32, 1024, 256]
